# revision 11
# baseline (speedup 1.0000x reference)
"""Causal attention head for Trainium2 — single-core, single-packed-input.

Reference computation (single head):
  q = x @ Wq.T + bq ; k = x @ Wk.T + bk ; v = x @ Wv.T + bv
  scores = (q @ k.T) / sqrt(D)  with causal mask, softmax, out = attn @ v

Why one core: per-iteration dispatch through the PJRT relay costs
~0.27 ms per argument buffer PER CORE (serialized), while the whole
problem is ~350 us of silicon on one core. 8-way sharding pays ~4.3 ms
of dispatch to save ~300 us of compute. One core + one packed input
+ one output is the fast configuration.

On-chip schedule (S=4096, D=2048, H=256):
  - All host-side layout prep (x/W transpose, bf16 cast, bias packing)
    is done in _shard_inputs; the device reads one [2049, 4864] bf16
    tensor: cols 0:4096 = x^T, then Wq^T | Wk^T | Wv^T, biases in the
    last row.
  - Projections stream x^T in 4 groups of 1024 rows: q^T/k^T stay in
    [head, seq] layout (so score matmuls contract head-dim on
    partitions); v in [row-chunk, 257] chunks with an all-ones column
    appended (softmax denominator accumulates in PSUM with the
    numerator, no separate reduction).
  - Scores are computed transposed ([key, query]) per 512-wide query
    block, exp'd on the scalar engine (|score| <= 0.66 so no max
    subtraction), causal-masked with affine_select on the <=4 diagonal
    chunks, then attn @ v accumulates per 128-row query chunk.
"""
import sys

sys.path.insert(0, "/opt/trn_rl_repo")

import numpy as np
import concourse.bass as bass
import concourse.mybir as mybir
from concourse import bacc, tile
from concourse.bass_utils import run_bass_kernel_spmd

F32 = mybir.dt.float32
BF16 = mybir.dt.bfloat16
F8 = mybir.dt.float8e4
N_CORES = 1
H = 256
HC = 2   # head-dim chunks of 128
WSCALE = 64.0  # host pre-scale on Wq/Wk/bq/bk so fp8e4 keeps W precision


def build_nc(S=4096, D=2048):
    KC = D // 128          # contraction chunks (16)
    NB = S // 512          # query blocks (8)
    NQ = S // 128          # query/key chunks (32)
    GROUPS = 4             # x streamed in GROUPS row-groups
    GR = S // GROUPS       # rows per group (1024)
    WCOL = S + 3 * H       # packed width: xT | wqT | wkT | wvT
    SCALE = 1.0 / float(np.sqrt(D))

    # scores psum carries WSCALE^2 * (q+bq)(k+bk); fold it into the exp scale
    ESCALE = SCALE / (WSCALE * WSCALE)

    nc = bacc.Bacc("TRN2", target_bir_lowering=False, debug=False,
                   enable_asserts=True, num_devices=1)
    inp = nc.dram_tensor("inp", [D + 1, WCOL], BF16, kind="ExternalInput")
    out = nc.dram_tensor("out", [S, H], F32, kind="ExternalOutput")

    with tile.TileContext(nc) as tc:
        with (
            tc.tile_pool(name="w", bufs=1) as wpool,
            tc.tile_pool(name="xg", bufs=2) as xgpool,
            tc.tile_pool(name="qkv", bufs=1) as qkvpool,
            tc.tile_pool(name="small", bufs=1) as small,
            tc.tile_pool(name="pt", bufs=NQ) as ptpool,
            tc.tile_pool(name="osb", bufs=2) as osbpool,
            tc.tile_pool(name="psp", bufs=3, space="PSUM") as psp,
            tc.tile_pool(name="pss", bufs=3, space="PSUM") as pss,
            tc.tile_pool(name="pso", bufs=2, space="PSUM") as pso,
        ):
            # ---- weights + biases -> SBUF ----
            wq_sb = wpool.tile([128, KC, H], BF16, tag="wq")
            wk_sb = wpool.tile([128, KC, H], BF16, tag="wk")
            wv_sb = wpool.tile([128, KC, H], BF16, tag="wv")
            wview = inp[0:D, :].rearrange("(kc p) c -> p kc c", p=128)
            nc.sync.dma_start(wq_sb[:], wview[:, :, S:S + H])
            nc.sync.dma_start(wk_sb[:], wview[:, :, S + H:S + 2 * H])
            nc.sync.dma_start(wv_sb[:], wview[:, :, S + 2 * H:S + 3 * H])
            # fp8 copies of (already 64x host-scaled) Wq/Wk for DoubleRow matmuls
            wq8 = wpool.tile([128, KC, H], F8, tag="wq8")
            wk8 = wpool.tile([128, KC, H], F8, tag="wk8")
            nc.vector.tensor_copy(wq8[:], wq_sb[:])
            nc.vector.tensor_copy(wk8[:], wk_sb[:])
            bqh = small.tile([128, HC], BF16, tag="bqh")
            bkh = small.tile([128, HC], BF16, tag="bkh")
            nc.sync.dma_start(bqh[:], inp[D, 0:H].rearrange("(hc p) -> p hc", p=128))
            nc.sync.dma_start(bkh[:], inp[D, H:2 * H].rearrange("(hc p) -> p hc", p=128))
            bq_sb = small.tile([128, HC], F32, tag="bq")
            bk_sb = small.tile([128, HC], F32, tag="bk")
            nc.vector.tensor_copy(bq_sb[:], bqh[:])
            nc.vector.tensor_copy(bk_sb[:], bkh[:])
            bv1_sb = small.tile([1, H + 1], BF16, tag="bv")
            nc.sync.dma_start(bv1_sb[:, 0:H], inp[D:D + 1, 2 * H:3 * H])
            nc.vector.memset(bv1_sb[:, H:H + 1], 1.0)
            ones_row = small.tile([1, 128], BF16, tag="ones")
            nc.vector.memset(ones_row[:], 1.0)

            # ---- projections, streaming x^T in 4 groups of GR rows ----
            qT_sb = qkvpool.tile([128, HC, S], F8, tag="qT")
            kT_sb = qkvpool.tile([128, HC, S], F8, tag="kT")
            v_sb = qkvpool.tile([128, NQ, H + 1], BF16, tag="v")
            xv = inp[0:D, :].rearrange("(kc p) c -> p kc c", p=128)
            DR = mybir.MatmulPerfMode.DoubleRow
            for g in range(GROUPS):
                xg = xgpool.tile([128, KC, GR], BF16, tag="xg")
                xg8 = xgpool.tile([128, KC, GR], F8, tag="xg8")
                for k0 in range(0, KC, 4):
                    nc.sync.dma_start(xg[:, k0:k0 + 4, :],
                                      xv[:, k0:k0 + 4, g * GR:(g + 1) * GR])
                    # second (casting) read feeds the fp8 q/k projections
                    nc.gpsimd.dma_start(xg8[:, k0:k0 + 4, :],
                                        xv[:, k0:k0 + 4, g * GR:(g + 1) * GR])
                # q^T / k^T for this group's columns ([head, seq] layout)
                for hc in range(HC):
                    for half in range(GR // 512):
                        c0 = half * 512
                        ps = psp.tile([128, 512], F32, tag="ps")
                        for k2 in range(KC // 2):
                            nc.tensor.matmul(
                                ps[:],
                                wq8[:, 2 * k2:2 * k2 + 2, hc * 128:(hc + 1) * 128],
                                xg8[:, 2 * k2:2 * k2 + 2, c0:c0 + 512],
                                start=(k2 == 0), stop=(k2 == KC // 2 - 1),
                                perf_mode=DR)
                        nc.vector.tensor_scalar_add(
                            qT_sb[:, hc, g * GR + c0:g * GR + c0 + 512],
                            ps[:], bq_sb[:, hc:hc + 1])
                        ps = psp.tile([128, 512], F32, tag="ps")
                        for k2 in range(KC // 2):
                            nc.tensor.matmul(
                                ps[:],
                                wk8[:, 2 * k2:2 * k2 + 2, hc * 128:(hc + 1) * 128],
                                xg8[:, 2 * k2:2 * k2 + 2, c0:c0 + 512],
                                start=(k2 == 0), stop=(k2 == KC // 2 - 1),
                                perf_mode=DR)
                        nc.vector.tensor_scalar_add(
                            kT_sb[:, hc, g * GR + c0:g * GR + c0 + 512],
                            ps[:], bk_sb[:, hc:hc + 1])
                # v rows for this group ([row, head] layout, ones col appended)
                for u in range(GR // 128):
                    m = g * (GR // 128) + u
                    ps = psp.tile([128, H + 1], F32, tag="ps")
                    nc.tensor.matmul(ps[:], ones_row[0:1, :], bv1_sb[:],
                                     start=True, stop=False)
                    for kc in range(KC):
                        nc.tensor.matmul(ps[:, 0:H],
                                         xg[:, kc, u * 128:(u + 1) * 128],
                                         wv_sb[:, kc, :],
                                         start=False, stop=(kc == KC - 1))
                    nc.vector.tensor_copy(v_sb[:, m, :], ps[:])

            # ---- causal attention, per 512-wide query block ----
            for b in range(NB):
                nm = 4 * b + 4      # key chunks this block sees
                pts = []
                for m in range(nm):
                    ps = pss.tile([128, 512], F32, tag="ps")
                    nc.tensor.matmul(
                        ps[:], kT_sb[:, :, m * 128:(m + 1) * 128],
                        qT_sb[:, :, b * 512:(b + 1) * 512],
                        start=True, stop=True, perf_mode=DR)
                    pt = ptpool.tile([128, 512], BF16, tag="pt")
                    nc.scalar.activation(pt[:], ps[:],
                                         mybir.ActivationFunctionType.Exp,
                                         scale=ESCALE)
                    if m >= 4 * b:
                        # keep iff (col within block) - key_row - (m-4b)*128 >= 0
                        nc.gpsimd.affine_select(
                            out=pt[:], in_=pt[:],
                            compare_op=mybir.AluOpType.is_ge,
                            fill=0.0, base=-(m - 4 * b) * 128,
                            pattern=[[1, 512]], channel_multiplier=-1)
                    pts.append(pt)
                for u in range(4):
                    Q = 4 * b + u   # global 128-row query chunk
                    po = pso.tile([128, H + 1], F32, tag="po")
                    for m in range(Q + 1):
                        nc.tensor.matmul(po[:],
                                         pts[m][:, u * 128:(u + 1) * 128],
                                         v_sb[:, m, :],
                                         start=(m == 0), stop=(m == Q))
                    recip = small.tile([128, 1], F32, tag=f"recip{u}")
                    nc.vector.reciprocal(recip[:], po[:, H:H + 1])
                    osb = osbpool.tile([128, H], F32, tag=f"osb{u}")
                    nc.vector.tensor_scalar_mul(osb[:], po[:, 0:H], recip[:])
                    nc.sync.dma_start(out[Q * 128:(Q + 1) * 128, :], osb[:])
    nc.compile()
    return nc


def _shard_inputs(marketStateBatch, Wq, bq, Wk, bk, Wv, bv):
    import ml_dtypes
    bf16 = ml_dtypes.bfloat16
    S, D = marketStateBatch.shape
    packed = np.zeros((D + 1, S + 3 * H), dtype=bf16)
    packed[0:D, 0:S] = marketStateBatch.T.astype(bf16)
    packed[0:D, S:S + H] = (Wq.T * WSCALE).astype(bf16)
    packed[0:D, S + H:S + 2 * H] = (Wk.T * WSCALE).astype(bf16)
    packed[0:D, S + 2 * H:S + 3 * H] = Wv.T.astype(bf16)
    packed[D, 0:H] = (bq * WSCALE).astype(bf16)
    packed[D, H:2 * H] = (bk * WSCALE).astype(bf16)
    packed[D, 2 * H:3 * H] = bv.astype(bf16)
    return [{"inp": packed}]


def _unshard(results, S):
    return results[0]["out"]


_NC_CACHE = {}


def kernel(marketStateBatch, Wq, bq, Wk, bk, Wv, bv):
    marketStateBatch = np.asarray(marketStateBatch, dtype=np.float32)
    S, D = marketStateBatch.shape
    key = (S, D)
    if key not in _NC_CACHE:
        _NC_CACHE[key] = build_nc(S, D)
    nc = _NC_CACHE[key]
    in_maps = _shard_inputs(marketStateBatch, np.asarray(Wq), np.asarray(bq),
                            np.asarray(Wk), np.asarray(bk),
                            np.asarray(Wv), np.asarray(bv))
    res = run_bass_kernel_spmd(nc, in_maps, core_ids=list(range(N_CORES)))
    return _unshard(res.results, S)
